# revision 14
# baseline (speedup 1.0000x reference)
"""Trainium2 Bass kernel v2 for nn_ConvTran (conv stem + eRPE transformer + GAP).

Sharding: pure data parallel, B=16 as 2 batch elems per core across 8 cores.

v2 vs baseline:
- bf16 matmuls throughout (PE 1 cyc/row vs fp32's 4).
- conv1 BN scale/bias folded into weights + ones-row (pure-Gelu batched
  activations), row-tiled 4x across the PE array.
- exp activations batched to FD=2048 over a 4-bank PSUM scores tile.
- phase order (conv for both batch elems, then the rest) + ln/exp-based
  LN rstd -> 2 act-table loads instead of 21.
- GAP accumulated in PSUM instead of DVE adds.
"""

import os
import numpy as np
import ml_dtypes

KDBG = bool(os.environ.get("KDBG"))
KDBG_G = int(os.environ.get("KDBG_G", "0"))

B, S, C_IN, E, H, DFF = 16, 1024, 4, 128, 8, 512
C1 = E * 4          # 512
DH = E // H         # 16
EPS = 1e-5
SCALE = float(E) ** -0.5
N_CORES = 8
NB = B // N_CORES   # 2 batch elems per core
NG = 2              # head groups of 4
SC = S // 128       # 8 s-chunks
JC = S // 128
F32 = np.float32
BF16 = ml_dtypes.bfloat16


class _Pack:
    """Column-packed [128, N] constant store."""

    def __init__(self, npdt):
        self.npdt = npdt
        self.cols = []
        self.index = {}
        self.n = 0

    def add(self, name, arr2d):
        a = np.zeros((128, arr2d.shape[1]), self.npdt)
        a[:arr2d.shape[0]] = arr2d
        self.index[name] = (self.n, arr2d.shape[1])
        self.cols.append(a)
        self.n += arr2d.shape[1]

    def finalize(self):
        return np.ascontiguousarray(np.concatenate(self.cols, axis=1))


def _host_prep(inp):
    f = lambda a: np.asarray(a, dtype=F32)
    pk = _Pack(F32)       # fp32 constants
    bk = _Pack(BF16)      # bf16 constants

    # conv1: weights*bnscale + bias row, replicated in 4 row-quadrants
    sA = f(inp["bn1_g"]) / np.sqrt(f(inp["bn1_v"]) + EPS)
    bA = (f(inp["conv1_b"]) - f(inp["bn1_m"])) * sA + f(inp["bn1_b"])
    w1 = f(inp["conv1_w"])[:, 0, 0, :] * sA[:, None]      # [C1, 8]
    w1q = np.zeros((128, 128), F32)
    for cc in range(4):
        w1q[32 * cc:32 * cc + 8, :] = w1[cc * 128:(cc + 1) * 128, :].T
        w1q[32 * cc + 8, :] = bA[cc * 128:(cc + 1) * 128]
    bk.add("w1q", w1q.astype(BF16))

    # conv2: [128(c1), 16(k=r*4+cc), 128(e)]
    w2 = f(inp["conv2_w"])[:, :, :, 0]                    # [E, C1, 4]
    w2cT = np.zeros((128, 16, 128), F32)
    for r in range(4):
        for cc in range(4):
            w2cT[:, r * 4 + cc, :] = w2[:, cc * 128:(cc + 1) * 128, r].T
    bk.add("w2cT", w2cT.reshape(128, 16 * 128).astype(BF16))
    sB = f(inp["bn2_g"]) / np.sqrt(f(inp["bn2_v"]) + EPS)
    pk.add("scaleB", sB[:, None].astype(F32))
    pk.add("biasB", ((f(inp["conv2_b"]) - f(inp["bn2_m"])) * sB
                     + f(inp["bn2_b"]))[:, None].astype(F32))

    # tAPE positional encoding, transposed [E, S]
    pos = np.arange(S, dtype=np.float64)[:, None]
    div = np.exp(np.arange(0, E, 2, dtype=np.float64) * (-np.log(10000.0) / E))
    ang = pos * div * (E / S)
    pe = np.zeros((S, E), np.float64)
    pe[:, 0::2] = np.sin(ang)
    pe[:, 1::2] = np.cos(ang)
    pk.add("peT", pe.astype(F32).T)

    # q/k weights, padded head layout [128, g*128 + 32c + dh]
    def pad_qk(w):
        w = f(w)
        wt = np.zeros((128, NG * 128), F32)
        for g in range(NG):
            for c in range(4):
                h = 4 * g + c
                wt[:, g * 128 + 32 * c:g * 128 + 32 * c + DH] = \
                    w[h * DH:(h + 1) * DH, :].T
        return wt
    bk.add("wqT", pad_qk(inp["wq"]).astype(BF16))
    bk.add("wkT", pad_qk(inp["wk"]).astype(BF16))
    bk.add("wvT", f(inp["wv"]).T.astype(BF16))

    bk.add("ffw1T", f(inp["ff_w1"]).T.astype(BF16))
    pk.add("ffb1", f(inp["ff_b1"]).reshape(4, 128).T.astype(F32))
    bk.add("ffw2T", (f(inp["ff_w2"]).T.reshape(4, 128, 128)
                     .transpose(1, 0, 2).reshape(128, 512)).astype(BF16))
    pk.add("ffb2", f(inp["ff_b2"])[:, None].astype(F32))

    m = np.arange(128)
    bk.add("bcast4", (m[None, :] // 32 == np.arange(4)[:, None]).astype(BF16))
    b128 = np.zeros((128, 128), F32)
    for c in range(4):
        b128[16 + 32 * c, 32 * c:32 * c + 32] = 1.0
    bk.add("bcast128", b128.astype(BF16))
    pk.add("ident", np.eye(128, dtype=F32))
    bk.add("identb", np.eye(128, dtype=BF16))
    pk.add("ones", np.ones((128, 1), F32))
    pk.add("eps", np.full((128, 1), EPS, F32))

    lnG = np.stack([f(inp["ln_attn_g"]), f(inp["ln1_g"]), f(inp["ln2_g"])])
    lnB = np.stack([f(inp["ln_attn_b"]), f(inp["ln1_b"]), f(inp["ln2_b"])])
    ln_identity = bool(np.allclose(lnG, 1.0) and np.allclose(lnB, 0.0))
    pk.add("lnG", np.broadcast_to(lnG.reshape(1, 3 * 128), (128, 384)).copy())
    pk.add("lnB", np.broadcast_to(lnB.reshape(1, 3 * 128), (128, 384)).copy())

    d = {"cpack": pk.finalize(), "bpack": bk.finalize()}

    # rel_bias Toeplitz blocks: ts2[jj, h, d, ii] = rel[128*(d-7)+ii-jj+1023, h]
    # (chunk-diff d-7 in [-7, 7]; bias@V block = ts2[:, h, d, :].T @ v_chunk)
    rel = f(inp["rel_bias"])                              # [2047, 8]
    jj = np.arange(128)[:, None, None, None]
    dd = np.arange(15)[None, None, :, None]
    ii = np.arange(128)[None, None, None, :]
    idx = 128 * (dd - 7) + ii - jj + 1023                 # [128,1,15,128]
    ts2 = rel[idx[:, 0], :]                               # [128,15,128,8]
    d["tstore"] = np.ascontiguousarray(
        ts2.transpose(0, 3, 1, 2).astype(BF16))           # [128,8,15,128]
    return d, (pk.index, bk.index), ln_identity


def _build_bass(index, npk, nbk, ln_identity):
    import concourse.bass as bass
    import concourse.bacc as bacc
    import concourse.tile as tile
    from concourse.tile import add_dep_helper
    import concourse.mybir as mybir

    dt = mybir.dt
    AF = mybir.ActivationFunctionType
    ALU = mybir.AluOpType
    pidx, bidx = index

    nc = bacc.Bacc("TRN2")

    # Chain all Act-engine instructions in emission order so the scheduler
    # cannot interleave Gelu (conv) with Exp/Ln (attention/LN) — keeps the
    # activation-table loads at 2 instead of ~14.
    _acts = []

    def ACT(*args, **kwargs):
        bi = nc.scalar.activation(*args, **kwargs)
        func = args[2]
        # Only chain table-anchored functions; Relu/Copy/Identity live in
        # every table set and may float freely in the schedule.
        if func in (AF.Gelu, AF.Exp, AF.Ln, AF.Sqrt):
            if _acts:
                add_dep_helper(bi.ins, _acts[-1].ins, sync=False,
                               reason="act table order")
            _acts.append(bi)
        return bi

    xin = nc.dram_tensor("rhs8q", [NB, 128, 4 * S], dt.bfloat16,
                         kind="ExternalInput")
    cpk_dr = nc.dram_tensor("cpack", [128, npk], dt.float32,
                            kind="ExternalInput")
    bpk_dr = nc.dram_tensor("bpack", [128, nbk], dt.bfloat16,
                            kind="ExternalInput")
    ts_dr = nc.dram_tensor("tstore", [128, H, 15, 128], dt.bfloat16,
                           kind="ExternalInput")
    yout = nc.dram_tensor("y", [NB, E], dt.float32, kind="ExternalOutput")
    dbg = {}
    if KDBG:
        for nm, shp in [("xsT", [128, S]), ("xpT", [128, S]),
                        ("qT0", [128, S]), ("kT0", [128, S]),
                        ("v", [128, SC * 128]), ("ut00", [128, 4 * 512]),
                        ("osb00", [128, 512]),
                        ("oatt0", [128, 128]), ("att0", [128, 128]),
                        ("ffT", [128, S])]:
            dbg[nm] = nc.dram_tensor("dbg_" + nm, shp, dt.float32,
                                     kind="ExternalOutput")

    with tile.TileContext(nc) as tc:
        import contextlib
        ctx = contextlib.ExitStack()
        with ctx:
            consts = ctx.enter_context(tc.tile_pool(name="consts", bufs=1))
            bpk = consts.tile([128, nbk], dt.bfloat16, tag="bpack")
            conv_end = bidx["wqT"][0]          # w1q + w2cT come first
            nc.sync.dma_start(out=bpk[:, 0:conv_end],
                              in_=bpk_dr[:, 0:conv_end])
            cpk = consts.tile([128, npk], dt.float32, tag="cpack")
            nc.sync.dma_start(out=cpk, in_=cpk_dr[:])
            nc.sync.dma_start(out=bpk[:, conv_end:],
                              in_=bpk_dr[:, conv_end:])
            ts_sb = consts.tile([128, H, 15, 128], dt.bfloat16,
                                tag="tstore")

            def C(name, rows=128):
                o, w = pidx[name]
                return cpk[0:rows, o:o + w]

            def Cb(name, rows=128):
                o, w = bidx[name]
                return bpk[0:rows, o:o + w]

            w1q_sb = Cb("w1q")
            w2cT_sb = Cb("w2cT").rearrange("p (k e) -> p k e", k=16)
            scaleB_sb, biasB_sb = C("scaleB"), C("biasB")
            peT_sb = C("peT")
            wqT_sb = Cb("wqT").rearrange("p (g e) -> p g e", g=NG)
            wkT_sb = Cb("wkT").rearrange("p (g e) -> p g e", g=NG)
            wvT_sb = Cb("wvT")
            ffw1T_sb = Cb("ffw1T")
            ffb1_sb = C("ffb1")
            ffw2T_sb = Cb("ffw2T").rearrange("p (k e) -> p k e", k=4)
            ffb2_sb = C("ffb2")
            bcast4_sb = Cb("bcast4", rows=4)
            ident = C("ident")
            ones_sb = C("ones")
            lnG_sb = C("lnG").rearrange("p (k e) -> p k e", k=3)
            lnB_sb = C("lnB").rearrange("p (k e) -> p k e", k=3)

            pers = ctx.enter_context(tc.tile_pool(name="pers", bufs=1))
            magic = pers.tile([128, SC, 1], dt.uint32, tag="magic",
                              name="magic")
            nc.vector.memset(magic, 0x5f3759df)

            xpT = [pers.tile([128, S], dt.bfloat16, tag=f"xpT{b}",
                             name=f"xpT{b}") for b in range(NB)]
            vball = pers.tile([128, JC, H, NB, DH], dt.bfloat16,
                              tag="vball", name="vball")
            obias = [pers.tile([128, SC, 128], dt.float32, tag=f"obias{b}",
                               name=f"obias{b}") for b in range(NB)]
            xsrc = [pers.tile([128, SC, 128], dt.float32, tag=f"xsrc{b}",
                              name=f"xsrc{b}") for b in range(NB)]

            # =========== PHASE C: conv stem (both batch elems; Gelu) ======
            with tc.tile_pool(name="h1", bufs=1) as h1p, \
                 tc.tile_pool(name="cmisc", bufs=2) as cmisc:
                h1s = [h1p.tile([128, 4, 4, S], dt.bfloat16, tag=f"h1_{b}",
                                name=f"h1_{b}") for b in range(NB)]
                # conv1: row-tiled 4x over cc; bias via ones row (9 taps);
                # 8-bank PSUM ring alternating 4-bank halves per round
                with tc.tile_pool(name="c1ps", bufs=1,
                                  space="PSUM") as c1psp:
                    for b in range(NB):
                        rhs8 = cmisc.tile([128, 4 * S], dt.bfloat16,
                                          tag="rhs8", name="rhs8")
                        nc.sync.dma_start(out=rhs8[:, 0:2 * S],
                                          in_=xin[b][:, 0:2 * S])
                        nc.sync.dma_start(out=rhs8[:, 2 * S:],
                                          in_=xin[b][:, 2 * S:])
                        ring = c1psp.tile([128, 2, 4, 512], dt.float32,
                                          tag="ring", name="ring")
                        for n in range(8):
                            hf = n % 2
                            for cc in range(4):
                                nc.tensor.matmul(
                                    ring[:, hf, cc, :],
                                    lhsT=w1q_sb[32 * cc:32 * cc + 9, :],
                                    rhs=rhs8[32 * cc:32 * cc + 9,
                                             n * 512:(n + 1) * 512],
                                    start=True, stop=True,
                                    tile_position=(32 * cc, 0))
                            r, sh = n // 2, n % 2
                            ACT(h1s[b][:, :, r, sh * 512:(sh + 1) * 512],
                                ring[:, hf, :, :], AF.Gelu)
                # conv2: accumulate 16 chunks per s-half
                with tc.tile_pool(name="c2ps", bufs=2,
                                  space="PSUM") as c2psp:
                    for b in range(NB):
                        h1 = h1s[b]
                        c2ps = c2psp.tile([128, 2, 512], dt.float32,
                                          tag="c2ps", name="c2ps")
                        for sh in range(2):
                            for k in range(16):
                                r, cc = k // 4, k % 4
                                nc.tensor.matmul(
                                    c2ps[:, sh, :], lhsT=w2cT_sb[:, k, :],
                                    rhs=h1[:, cc, r, sh * 512:(sh + 1) * 512],
                                    start=(k == 0), stop=(k == 15),
                                    skip_group_check=True)
                        xsT = cmisc.tile([128, S], dt.float32, tag="xsT",
                                         name="xsT")
                        ACT(xsT.rearrange("p (a c) -> p a c", a=2), c2ps,
                            AF.Gelu, bias=biasB_sb, scale=scaleB_sb)
                        if KDBG and b == 0:
                            nc.sync.dma_start(out=dbg["xsT"][:], in_=xsT)
                        nc.vector.tensor_tensor(xpT[b], xsT, peT_sb,
                                                op=ALU.add)
                        if KDBG and b == 0:
                            xpc = cmisc.tile([128, S], dt.float32, tag="xpc",
                                             name="xpc")
                            nc.vector.tensor_copy(out=xpc, in_=xpT[b])
                            nc.sync.dma_start(out=dbg["xpT"][:], in_=xpc)
                        for sc in range(SC):
                            ps = c2psp.tile([128, 128], dt.float32,
                                            tag="tps", name="tps")
                            nc.tensor.transpose(
                                ps, xsT[:, sc * 128:(sc + 1) * 128], ident)
                            nc.vector.tensor_copy(out=xsrc[b][:, sc, :],
                                                  in_=ps)

            # tstore DMA issued after the conv-phase DMAs so it doesn't
            # delay conv weights on the queue (first use is in attention)
            nc.sync.dma_start(out=ts_sb, in_=ts_dr[:])

            # =========== PHASE A: attention + tail (exp/ln table) =========
            scp = ctx.enter_context(
                tc.tile_pool(name="scp", bufs=2, space="PSUM"))
            pvp = ctx.enter_context(
                tc.tile_pool(name="pvp", bufs=1, space="PSUM"))
            biasp = ctx.enter_context(
                tc.tile_pool(name="biasp", bufs=1, space="PSUM"))
            msp = ctx.enter_context(
                tc.tile_pool(name="msp", bufs=2, space="PSUM"))
            utp = ctx.enter_context(tc.tile_pool(name="utp", bufs=3))
            qkv = ctx.enter_context(tc.tile_pool(name="qkv", bufs=2))
            att_p = ctx.enter_context(tc.tile_pool(name="attp", bufs=2))
            sm = ctx.enter_context(tc.tile_pool(name="sm", bufs=2))

            oatts = []
            for b in range(NB):
                # ---------- QKV ----------
                qT, kT = [], []
                for g in range(NG):
                    qt = qkv.tile([128, S], dt.bfloat16, tag=f"qt{g}",
                                  name=f"qt{g}")
                    kt = qkv.tile([128, S], dt.bfloat16, tag=f"kt{g}",
                                  name=f"kt{g}")
                    for sh in range(2):
                        ps = msp.tile([128, 512], dt.float32, tag="ms",
                                      name="msq")
                        nc.tensor.matmul(ps, lhsT=wqT_sb[:, g, :],
                                         rhs=xpT[b][:, sh * 512:(sh + 1) * 512],
                                         start=True, stop=True)
                        nc.vector.tensor_copy(
                            out=qt[:, sh * 512:(sh + 1) * 512], in_=ps)
                        ps2 = msp.tile([128, 512], dt.float32, tag="ms",
                                       name="msk")
                        nc.tensor.matmul(ps2, lhsT=wkT_sb[:, g, :],
                                         rhs=xpT[b][:, sh * 512:(sh + 1) * 512],
                                         start=True, stop=True)
                        nc.vector.tensor_copy(
                            out=kt[:, sh * 512:(sh + 1) * 512], in_=ps2)
                    if KDBG and b == 0 and g == KDBG_G:
                        qtc = sm.tile([128, S], dt.float32, tag="qtc",
                                      bufs=1, name="qtc")
                        nc.vector.tensor_copy(out=qtc, in_=qt)
                        nc.sync.dma_start(out=dbg["qT0"][:], in_=qtc)
                        nc.vector.tensor_copy(out=qtc, in_=kt)
                        nc.sync.dma_start(out=dbg["kT0"][:], in_=qtc)
                    qT.append(qt)
                    kT.append(kt)
                for sc in range(SC):
                    ps = msp.tile([128, 128], dt.float32, tag="ms", name="msv")
                    nc.tensor.matmul(ps,
                                     lhsT=xpT[b][:, sc * 128:(sc + 1) * 128],
                                     rhs=wvT_sb, start=True, stop=True)
                    nc.vector.tensor_copy(
                        out=vball[:, sc, :, b, :],
                        in_=ps.rearrange("p (h d) -> p h d", h=H))
                if KDBG and b == 0:
                    vc = sm.tile([128, SC * 128], dt.float32, tag="vc",
                                 bufs=1, name="vc")
                    nc.vector.tensor_copy(
                        out=vc.rearrange("p (jc h d) -> p jc h d",
                                         jc=JC, h=H),
                        in_=vball[:, :, :, b, :])
                    nc.sync.dma_start(out=dbg["v"][:], in_=vc)
                VO = []
                vsrc = vball[:, :, :, b, :]
                for g in range(NG):
                    vo = qkv.tile([128, JC, 4, 32], dt.bfloat16, tag=f"vo{g}",
                                  name=f"vo{g}")
                    nc.vector.tensor_copy(out=vo[:, :, :, 0:DH],
                                          in_=vsrc[:, :, 4 * g:4 * g + 4, :])
                    nc.vector.memset(vo[:, :, :, DH:DH + 1], 1.0)
                    VO.append(vo)

                oatt = att_p.tile([128, SC, 128], dt.float32, tag="oatt",
                                  name="oatt")
                # ---------- attention core ----------
                for g in range(NG):
                    for ih in range(2):
                        i0 = ih * 512
                        pv = pvp.tile([128, 512], dt.float32, tag="pv",
                                      name="pv")
                        for jc in range(JC):
                            for hf in range(2):
                                st = scp.tile([128, 2, 512], dt.float32,
                                              tag="st", name="st")
                                for ci in range(2):
                                    c = 2 * hf + ci
                                    nc.tensor.matmul(
                                        st[:, ci, :],
                                        lhsT=kT[g][32 * c:32 * c + DH,
                                                   jc * 128:(jc + 1) * 128],
                                        rhs=qT[g][32 * c:32 * c + DH,
                                                  i0:i0 + 512],
                                        start=True, stop=True,
                                        tile_position=(32 * c, 0))
                                ut = utp.tile([128, 2, 512], dt.bfloat16,
                                              tag="ut", name="ut")
                                ACT(ut, st, AF.Exp, scale=SCALE)
                                if (KDBG and b == 0 and g == KDBG_G
                                        and ih == 0 and jc == 0):
                                    utc = sm.tile([128, 2 * 512], dt.float32,
                                                  tag="utc", bufs=1,
                                                  name="utc")
                                    nc.vector.tensor_copy(
                                        out=utc,
                                        in_=ut.rearrange("p a b -> p (a b)"))
                                    nc.sync.dma_start(
                                        out=dbg["ut00"][:, hf * 1024:
                                                        (hf + 1) * 1024],
                                        in_=utc)
                                for ci in range(2):
                                    c = 2 * hf + ci
                                    nc.tensor.matmul(
                                        pv[32 * c:32 * c + DH + 1, :],
                                        lhsT=VO[g][:, jc, c, 0:DH + 1],
                                        rhs=ut[:, ci, :],
                                        start=(jc == 0),
                                        stop=(jc == JC - 1),
                                        skip_group_check=True,
                                        tile_position=(0, 32 * c))
                        # copy pv out of PSUM right away so the 1-bank
                        # pool doesn't serialize the next (g, ih) iteration
                        osb = sm.tile([128, 512], dt.float32, tag="osb",
                                      name="osb")
                        nc.vector.tensor_copy(out=osb, in_=pv)
                        rs = sm.tile([4, 512], dt.float32, tag="rs",
                                     name="rs")
                        nc.sync.dma_start(
                            out=rs,
                            in_=osb.rearrange(
                                "(a w) m -> a w m", a=4)[:, DH, :])
                        rr = sm.tile([4, 512], dt.bfloat16, tag="rr",
                                     name="rr")
                        with nc.allow_low_precision(
                                reason="softmax denom reciprocal to bf16"):
                            nc.vector.reciprocal(out=rr, in_=rs)
                        rsb = msp.tile([128, 512], dt.float32, tag="ms",
                                       name="rsb")
                        nc.tensor.matmul(rsb, lhsT=bcast4_sb, rhs=rr,
                                         start=True, stop=True)
                        nc.vector.tensor_tensor(osb, osb, rsb, op=ALU.mult)
                        if KDBG and b == 0 and g == KDBG_G and ih == 0:
                            nc.sync.dma_start(out=dbg["osb00"][:], in_=osb)
                        for ic in range(4):
                            ps = msp.tile([128, 128], dt.float32, tag="ms",
                                          name="mst")
                            nc.tensor.transpose(
                                ps, osb[:, ic * 128:(ic + 1) * 128], ident)
                            sc = ih * 4 + ic
                            psr = ps.rearrange("p (c m) -> p c m", c=4)
                            nc.vector.tensor_copy(
                                out=oatt[:, sc, :].rearrange(
                                    "p (h d) -> p h d",
                                    h=H)[:, 4 * g:4 * g + 4, :],
                                in_=psr[:, :, 0:DH])

                oatts.append(oatt)

            # ---------- bias@V via Toeplitz blocks (both b at once) ------
            # obias[b][i, sc_i, (h d)] = sum_j rel[i-j+1023, h] * v[b, j, h, d]
            for a in range(SC):
                bps = biasp.tile([128, H, NB * DH], dt.float32, tag="bps",
                                 name="bps")
                for h in range(H):
                    for bj in range(JC):
                        nc.tensor.matmul(
                            bps[:, h, :],
                            lhsT=ts_sb[:, h, a - bj + 7, :],
                            rhs=vball[:, bj, h, :, :],
                            start=(bj == 0), stop=(bj == JC - 1),
                            skip_group_check=True)
                for b in range(NB):
                    nc.vector.tensor_copy(
                        out=obias[b][:, a, :].rearrange(
                            "p (h d) -> p h d", h=H),
                        in_=bps.rearrange(
                            "p h (b2 d) -> p h b2 d", b2=NB)[:, :, b, :])

            # ---------- LN / FFN / GAP (s-half pipelined over b) ----------
            def layer_norm_half(dst_tile, src_tile, k, s0):
                """LN over E for chunks [s0, s0+4) of a [128, SC, 128] tile."""
                stats = sm.tile([128, 4, 6], dt.float32, tag="stats",
                                name="stats")
                mvall = sm.tile([128, 4, 2], dt.float32, tag="mvall",
                                name="mvall")
                for i in range(4):
                    nc.vector.bn_stats(out=stats[:, i, :],
                                       in_=src_tile[:, s0 + i, :])
                    nc.vector.bn_aggr(out=mvall[:, i, :],
                                      in_=stats[:, i, :])
                # rstd = (var+eps)^-0.5 via quake-seed + 2 Newton iters
                # (DVE-only: keeps Ln/Sqrt out of the Act table stream)
                veps = sm.tile([128, 4, 1], dt.float32, tag="veps",
                               name="veps")
                nc.vector.tensor_scalar(veps, mvall[:, :, 1:2], EPS, None,
                                        ALU.add)
                halfv = sm.tile([128, 4, 1], dt.float32, tag="halfv",
                                name="halfv")
                nc.vector.tensor_scalar(halfv, veps, 0.5, None, ALU.mult)
                rstd = sm.tile([128, 4, 1], dt.float32, tag="rstd",
                               name="rstd")
                yi = rstd.bitcast(dt.uint32)
                nc.vector.tensor_scalar(yi, veps.bitcast(dt.uint32), 1,
                                        None, ALU.logical_shift_right)
                nc.vector.tensor_tensor(yi, magic[:, 0:4, :], yi,
                                        op=ALU.subtract)
                nt = sm.tile([128, 4, 1], dt.float32, tag="nt", name="nt")
                for _ in range(1):
                    nc.vector.tensor_tensor(nt, rstd, rstd, op=ALU.mult)
                    nc.vector.tensor_tensor(nt, nt, halfv, op=ALU.mult)
                    nc.vector.tensor_scalar(nt, nt, -1.0, 1.5,
                                            ALU.mult, ALU.add)
                    nc.vector.tensor_tensor(rstd, rstd, nt, op=ALU.mult)
                for i in range(4):
                    nc.vector.tensor_scalar(
                        dst_tile[:, s0 + i, :], src_tile[:, s0 + i, :],
                        mvall[:, i, 0:1], rstd[:, i, 0:1],
                        ALU.subtract, ALU.mult)
                if not ln_identity:
                    for i in range(4):
                        nc.vector.tensor_tensor(
                            dst_tile[:, s0 + i, :], dst_tile[:, s0 + i, :],
                            lnG_sb[:, k, :], op=ALU.mult)
                        nc.vector.tensor_tensor(
                            dst_tile[:, s0 + i, :], dst_tile[:, s0 + i, :],
                            lnB_sb[:, k, :], op=ALU.add)

            atts, attTs, hrelus, gaps = [], [], [], []
            for b in range(NB):
                atts.append(att_p.tile([128, SC, 128], dt.float32,
                                       tag=f"att{b}", name=f"att{b}"))
                attTs.append(att_p.tile([128, S], dt.bfloat16,
                                        tag=f"attT{b}", name=f"attT{b}"))
                hrelus.append(att_p.tile([128, 4, S], dt.bfloat16,
                                         tag=f"hr{b}", name=f"hr{b}"))
                gaps.append(sm.tile([128, 2], dt.float32, tag=f"gap{b}",
                                    bufs=1, name=f"gap{b}"))

            def tail_half(b, hf):
                oatt, att, attT = oatts[b], atts[b], attTs[b]
                hrelu = hrelus[b]
                s0, i0 = hf * 4, hf * 512
                nc.vector.tensor_tensor(
                    oatt[:, s0:s0 + 4, :].rearrange("p a b -> p (a b)"),
                    oatt[:, s0:s0 + 4, :].rearrange("p a b -> p (a b)"),
                    obias[b][:, s0:s0 + 4, :].rearrange("p a b -> p (a b)"),
                    op=ALU.add)
                if KDBG and b == 0 and hf == 0:
                    nc.sync.dma_start(out=dbg["oatt0"][:],
                                      in_=oatt[:, 0, :])
                layer_norm_half(oatt, oatt, 0, s0)
                nc.vector.tensor_tensor(
                    oatt[:, s0:s0 + 4, :].rearrange("p a b -> p (a b)"),
                    oatt[:, s0:s0 + 4, :].rearrange("p a b -> p (a b)"),
                    xsrc[b][:, s0:s0 + 4, :].rearrange("p a b -> p (a b)"),
                    op=ALU.add)
                layer_norm_half(att, oatt, 1, s0)
                for i in range(4):
                    sc = s0 + i
                    ps = msp.tile([128, 128], dt.float32, tag="ms",
                                  name="msat")
                    nc.tensor.transpose(ps, att[:, sc, :], ident)
                    nc.vector.tensor_copy(
                        out=attT[:, sc * 128:(sc + 1) * 128], in_=ps)
                if KDBG and b == 0 and hf == 0:
                    nc.sync.dma_start(out=dbg["att0"][:], in_=att[:, 0, :])
                for fc in range(4):
                    f1ps = msp.tile([128, 512], dt.float32, tag="ms",
                                    name="f1ps")
                    nc.tensor.matmul(
                        f1ps,
                        lhsT=ffw1T_sb[:, fc * 128:(fc + 1) * 128],
                        rhs=attT[:, i0:i0 + 512],
                        start=True, stop=True)
                    ACT(hrelu[:, fc, i0:i0 + 512], f1ps, AF.Relu,
                        bias=ffb1_sb[:, fc:fc + 1])
                ps = msp.tile([128, 512], dt.float32, tag="ms", name="msf2")
                for fc in range(4):
                    nc.tensor.matmul(
                        ps, lhsT=ffw2T_sb[:, fc, :],
                        rhs=hrelu[:, fc, i0:i0 + 512],
                        start=(fc == 0), stop=(fc == 3))
                ffT = sm.tile([128, 512], dt.float32, tag="ffT", name="ffT")
                nc.vector.tensor_scalar(ffT, ps, ffb2_sb, None, ALU.add)
                if KDBG and b == 0:
                    nc.sync.dma_start(out=dbg["ffT"][:, i0:i0 + 512],
                                      in_=ffT)
                l2in = sm.tile([128, 4, 128], dt.float32, tag="l2in",
                               name="l2in")
                for i in range(4):
                    ps = msp.tile([128, 128], dt.float32, tag="ms",
                                  name="msft")
                    nc.tensor.transpose(ps, ffT[:, i * 128:(i + 1) * 128],
                                        ident)
                    nc.vector.tensor_tensor(l2in[:, i, :],
                                            att[:, s0 + i, :],
                                            ps, op=ALU.add)
                layer_norm_half(l2in, l2in, 2, 0)
                gp = msp.tile([128, 1], dt.float32, tag="ms", name="msg")
                for i in range(4):
                    nc.tensor.matmul(gp, lhsT=l2in[:, i, :], rhs=ones_sb,
                                     start=(i == 0), stop=(i == 3),
                                     skip_group_check=True)
                nc.vector.tensor_copy(out=gaps[b][:, hf:hf + 1], in_=gp)

            for hf in range(2):
                for b in range(NB):
                    tail_half(b, hf)
            for b in range(NB):
                ob = sm.tile([128, 1], dt.float32, tag="ob", name="ob")
                nc.vector.tensor_tensor(ob, gaps[b][:, 0:1],
                                        gaps[b][:, 1:2], op=ALU.add)
                nc.scalar.mul(ob, ob, 1.0 / S)
                nc.sync.dma_start(out=yout[b, :, None], in_=ob)

    nc.compile()
    return nc


_CACHE = {}


def _build(inputs):
    host, index, ln_identity = _host_prep(inputs)
    key = (ln_identity, host["cpack"].shape[1], host["bpack"].shape[1],
           KDBG, KDBG_G)
    if key not in _CACHE:
        _CACHE[key] = _build_bass(index, host["cpack"].shape[1],
                                  host["bpack"].shape[1], ln_identity)
    return _CACHE[key], host


def kernel(**inputs):
    inputs = {k: np.asarray(v) for k, v in inputs.items()}
    nc, host = _build(inputs)
    from concourse.bass_utils import run_bass_kernel_spmd
    in_maps = _make_in_maps(inputs, host)
    res = run_bass_kernel_spmd(nc, in_maps, list(range(N_CORES)))
    if KDBG:
        kernel.dbg = res.results[0]
    outs = [res.results[c]["y"] for c in range(N_CORES)]
    return np.concatenate(outs, axis=0).astype(F32)


def _make_in_maps(inputs, host):
    x = np.asarray(inputs["x"], dtype=F32)                # [B, S, 4]
    xpad = np.zeros((B, S + 7, C_IN), F32)
    xpad[:, 3:S + 3, :] = x
    rhs8 = np.empty((B, 8, C_IN, S), F32)
    for t in range(8):
        rhs8[:, t] = xpad[:, t:t + S, :].transpose(0, 2, 1)
    rhs8 = rhs8.reshape(B, 8, C_IN * S)
    # replicate in 4 row-quadrants + ones row at 32q+8 (conv1 bias)
    rhs8q = np.zeros((B, 128, C_IN * S), BF16)
    for q in range(4):
        rhs8q[:, 32 * q:32 * q + 8] = rhs8.astype(BF16)
        rhs8q[:, 32 * q + 8] = 1.0
    in_maps = []
    for core in range(N_CORES):
        m = {"rhs8q": np.ascontiguousarray(rhs8q[core * NB:(core + 1) * NB])}
        m.update(host)
        in_maps.append(m)
    return in_maps


def build(inputs):
    inputs = {k: np.asarray(v) for k, v in inputs.items()}
    nc, host = _build(inputs)
    return nc, _make_in_maps(inputs, host)
